# revision 32
# baseline (speedup 1.0000x reference)
"""Trainium2 Bass kernel for nn_MultiHeadDuelingDQN (8-core SPMD), v9.

Model (B=256, STATE=26240, H=512, R=4000, N=64 heads, M=10):
    h  = relu(relu(x@W1+b1)@W2+b2)
    q_cache = h@Wvc+bvc + (h@Wac+bac) - mean_R(h@Wac+bac)
    q_assoc = per-head dueling over M (local means)
    q_rec   = S - mean_R(S),  S = sum_n (h@Wru[n]+bru[n])   [exact rewrite:
              rec_global has zero row-mean so the second mean is a no-op]

Design (v9; v8 was ~130us/core intrinsic + launch skew, this is ~85us):
  - Wru (the dominant 262MB) streams in fp8e4 (TRN E4M3, max +-240) -- HALF
    of v8's bf16 bytes. Accuracy is preserved by error-feedback quantization
    across the 64 heads (head n's rounding residual carries into head n+1),
    so the on-device head-SUM keeps ~bf16 accuracy (measured +3.8e-3 rel vs
    naive fp8's +3.0e-2) while every stored head remains a faithful ~1-ulp
    fp8 image of Wru[n]. All 64 heads are still read and summed on-device.
  - The 64-head pre-sum runs on the PE (v8 used a DVE add tree at ~108us
    busy): stream blocks [p=(n4,h32), (g8, i2, r500)] are contracted with a
    constant 0/1 selector lhsT [128, 2, 128] via fp8 DoubleRow matmuls --
    8 heads per MM, 32 MMs per kc accumulating into one PSUM bank, one
    scaled ACT copy out. DVE is nearly free; PE adds ~25us under the
    ~65us stream.
  - fc1 contraction-sharded; partials combined with ONE bf16 AllReduce.
    x+W1 ride the sync queue AHEAD of the wru stream so every core's AR
    doorbell rings before the runtime launch-barrier releases; the AR
    transfer then completes ~37us post-release, hidden under the stream.
  - Row-sums for the full-R means come from tiny side matmuls (ones@vb,
    hT@v4 for S; ones@bacsum, hT@vwac for adv_c) so the single tail
    AllGather triggers ~1.5us after the last stream byte; psC/psS/psA all
    run during the collective. qr on DVE from PSUM, qc on ACT, per-half
    output DMAs split across the sync/scalar queues.
  - relu on DVE (tensor_scalar ADD,MAX 4x mode), per-kc sliced AR readback
    with kc-outer fc2, assoc-head means via one 3D DVE reduce.

kernel(**inputs) takes full unsharded fp32 inputs, returns full [256, 8640].
"""
import os
os.environ.setdefault("NEURON_RT_DBG_RDH_CC", "0")

import numpy as np
import ml_dtypes

import concourse.bass as bass
import concourse.mybir as mybir
import concourse.tile as tile
from concourse import bacc
from concourse import bass_utils
from concourse.bass import ts

NC = 8
B, H, STATE, R, NH, M = 256, 512, 26240, 4000, 64, 10
KPC_RAW = STATE // NC          # 3280
KCH = 26                       # k-chunks of 128 per core (padded)
KPC = KCH * 128                # 3328
RPC = R // NC                  # 500
RP = 500                       # r-slab (16B-stride rule applies to weights AP only)
HPC = NH // NC                 # 8 heads per core
AUG = HPC * (M + 1) + 1        # 89 = [8x(10 adv + 1 val)] + value_c
QS = 1024.0                    # fp8 quantization scale for Wru
F32 = mybir.dt.float32
BF16 = mybir.dt.bfloat16
FP8 = mybir.dt.float8e4
RELU = mybir.ActivationFunctionType.Relu
COPY = mybir.ActivationFunctionType.Copy
IDENT = mybir.ActivationFunctionType.Identity
ADD = mybir.AluOpType.add
DR = mybir.MatmulPerfMode.DoubleRow
BF = ml_dtypes.bfloat16
F8 = ml_dtypes.float8_e4m3

# x/w1 interleave groups for fc1 pipelining
FCG = [(0, 5), (5, 5), (10, 5), (15, 5), (20, 6)]


def build_program(wru_bufs=8):
    nc = bacc.Bacc("TRN2", target_bir_lowering=False, debug=False, num_devices=NC)

    # ---- per-core I/O (all host-packed to exact SBUF images) ----
    xt = nc.dram_tensor("xt", [128, KCH * B], BF16, kind="ExternalInput").ap()
    w1 = nc.dram_tensor("w1", [128, KCH * H], BF16, kind="ExternalInput").ap()
    b1p = nc.dram_tensor("b1p", [128, 4], F32, kind="ExternalInput").ap()
    w2p = nc.dram_tensor("w2p", [128, 4 * H], BF16, kind="ExternalInput").ap()
    b2p = nc.dram_tensor("b2p", [128, 4], F32, kind="ExternalInput").ap()
    wacp = nc.dram_tensor("wacp", [128, 4 * RPC], BF16, kind="ExternalInput").ap()
    bacp = nc.dram_tensor("bacp", [1, RPC], BF16, kind="ExternalInput").ap()
    # wru stream blocks: [kc, hb, p=(n4*32+h32), (g*2+i)*RP + r] =
    #   fp8_fb(Wru[8g+4i+n4, kc*128+hb*32+h32, r0+r]) * QS   (r<RPC; 0-pad to RP)
    wrup = nc.dram_tensor("wrup", [4, 4, 128, 16 * RP], FP8,
                          kind="ExternalInput").ap()
    # selector lhsT per h32-block: [p, hb*256 + i*128 + m] = (m == hb*32 + p%32)
    selp = nc.dram_tensor("selp", [128, 4 * 256], FP8, kind="ExternalInput").ap()
    brup = nc.dram_tensor("brup", [NH, RPC], BF16, kind="ExternalInput").ap()
    augp = nc.dram_tensor("augp", [128, 4 * AUG], BF16, kind="ExternalInput").ap()
    augb = nc.dram_tensor("augb", [1, AUG], BF16, kind="ExternalInput").ap()

    out_cache = nc.dram_tensor("out_cache", [B, RPC], BF16, kind="ExternalOutput").ap()
    out_rec = nc.dram_tensor("out_rec", [B, RPC], BF16, kind="ExternalOutput").ap()
    out_assoc = nc.dram_tensor("out_assoc", [B, HPC * M], F32, kind="ExternalOutput").ap()

    with tile.TileContext(nc) as tc, \
         nc.allow_low_precision(reason="fp8/bf16 stream; gate is 2e-2"):
        with (
            tc.tile_pool(name="cst", bufs=1) as cst,
            tc.tile_pool(name="sb", bufs=1) as sb,
            tc.tile_pool(name="wrupool", bufs=wru_bufs) as wrupool,
            tc.tile_pool(name="psfc", bufs=4, space="PSUM") as psfc,
            tc.tile_pool(name="psw", bufs=1, space="PSUM") as psw,
            tc.tile_pool(name="pshs", bufs=1, space="PSUM") as pshs,
            tc.tile_pool(name="pss", bufs=2, space="PSUM") as pss,
            tc.tile_pool(name="dram", bufs=1, space="DRAM") as dram,
        ):
            ones1 = cst.tile([1, 128], BF16, tag="ones1")
            nc.vector.memset(ones1, 1.0)
            ones64 = cst.tile([64, 128], BF16, tag="ones64")
            nc.vector.memset(ones64, 1.0)

            # ~7us of dummy matmuls: HAM releases the PE clock gate before
            # fc1 arrives, and the PE stays busy until x/w1 land
            warm_ps = psw.tile([128, 128], F32, tag="wide", name="warm_ps")
            for i in range(64):
                nc.tensor.matmul(warm_ps, ones64, ones64,
                                 start=(i == 0), stop=(i == 63))

            # ---------- scalar queue: fc1 inputs (interleaved per kc-group),
            # then the small head tensors ----------
            # separate tiles per group: same-tile slice-DMAs get serialized
            # with completion round-trips by the tile dependency tracker
            xg, w1g = [], []
            for gi, (base, L) in enumerate(FCG):
                tx = cst.tile([128, L * B], BF16, tag=f"xg{gi}", name=f"xg{gi}")
                nc.sync.dma_start(tx, xt[:, base * B:(base + L) * B])
                xg.append(tx)
                tw = cst.tile([128, L * H], BF16, tag=f"w1g{gi}", name=f"w1g{gi}")
                nc.sync.dma_start(tw, w1[:, base * H:(base + L) * H])
                w1g.append(tw)
            selsb = cst.tile([128, 4 * 256], FP8, tag="selsb")
            nc.scalar.dma_start(selsb, selp)
            b1sb = cst.tile([128, 4], F32, tag="b1sb")
            nc.scalar.dma_start(b1sb, b1p)
            b2sb = cst.tile([128, 4], F32, tag="b2sb")
            nc.scalar.dma_start(b2sb, b2p)
            w2sb = cst.tile([128, 4 * H], BF16, tag="w2sb")
            nc.scalar.dma_start(w2sb, w2p)
            wacsb = cst.tile([128, 4 * RPC], BF16, tag="wacsb")
            nc.scalar.dma_start(wacsb, wacp)
            bacsb = cst.tile([1, RPC], BF16, tag="bacsb")
            nc.scalar.dma_start(bacsb, bacp)
            # column-sums of Wac/bac (feed the adv_c row-sum matmuls so the
            # tail collective doesn't wait on the psC pipeline)
            vwac = sb.tile([128, 4], BF16, tag="vwac")
            for kc in range(4):
                nc.vector.tensor_reduce(vwac[:, kc:kc + 1],
                                        wacsb[:, ts(kc, RPC)],
                                        axis=mybir.AxisListType.X, op=ADD)
            bacsum = sb.tile([1, 1], BF16, tag="bacsum")
            nc.vector.tensor_reduce(bacsum, bacsb,
                                    axis=mybir.AxisListType.X, op=ADD)
            augsb = cst.tile([128, 4 * AUG], BF16, tag="augsb")
            nc.scalar.dma_start(augsb, augp)
            augbsb = cst.tile([1, AUG], BF16, tag="augbsb")
            nc.scalar.dma_start(augbsb, augb)
            brusb = cst.tile([64, RPC], BF16, tag="brusb")
            nc.scalar.dma_start(brusb, brup)
            # bru row-sum (feeds the S row-sum matmuls at the tail)
            vb = sb.tile([64, 1], BF16, tag="vb")
            nc.vector.tensor_reduce(vb, brusb, axis=mybir.AxisListType.X, op=ADD)

            sel4 = selsb.rearrange("p (hb i m) -> p hb i m", hb=4, i=2)

            # ---------- sync queue: the wru fp8 stream ----------
            # one 1MB DMA per (kc, hb) block; the last block lands as 4
            # quarter-DMAs so the tail waits on ~256KB, not 1MB
            wt = {}
            for kc in range(4):
                for hb in range(4):
                    if kc == 3 and hb == 3:
                        # last block as 4 separate quarter-tiles so the tail
                        # waits on 256KB (and quarters aren't serialized)
                        wq4 = []
                        for q in range(4):
                            t = wrupool.tile([128, 4 * RP], FP8, tag="wruq",
                                             name=f"wruq{q}")
                            nc.sync.dma_start(
                                t, wrup[kc, hb][:, q * 4 * RP:(q + 1) * 4 * RP])
                            wq4.append(t.rearrange("p (g i r) -> p g i r",
                                                   i=2, r=RP))
                        wt[(kc, hb)] = ("Q", wq4)
                    else:
                        t = wrupool.tile([128, 16 * RP], FP8, tag="wru",
                                         name=f"wru_{kc}_{hb}")
                        nc.sync.dma_start(t, wrup[kc, hb])
                        wt[(kc, hb)] = t.rearrange("p (g i r) -> p g i r",
                                                   i=2, r=RP)

            # ---------- fc1: h1T[ht] = sum_k W1[k, ht]*xT[k, b] ----------
            ps1 = [psfc.tile([128, B], F32, tag="fc", name=f"ps1_{ht}")
                   for ht in range(4)]
            for gi, (base, L) in enumerate(FCG):
                for j in range(L):
                    kc = base + j
                    for ht in range(4):
                        nc.tensor.matmul(
                            ps1[ht],
                            w1g[gi][:, j * H + ht * 128:j * H + (ht + 1) * 128],
                            xg[gi][:, ts(j, B)],
                            start=(kc == 0), stop=(kc == KCH - 1))
            h1loc = sb.tile([128, 4, B], BF16, tag="h1loc")
            for ht in range(4):
                nc.scalar.copy(h1loc[:, ht, :], ps1[ht])

            # ---------- fc1 cross-core reduction: ONE AllReduce (bf16) ----
            ar_din = dram.tile([128, 4, B], BF16, tag="ar_din")
            ar_dout = dram.tile([128, 4, B], BF16, tag="ar_dout",
                                addr_space="Shared")
            nc.scalar.dma_start(ar_din, h1loc)
            nc.gpsimd.collective_compute(
                "AllReduce", ADD,
                replica_groups=[list(range(NC))],
                ins=[ar_din.opt()], outs=[ar_dout.opt()],
            )

            # ---------- PE head-sum: acc[kc] = QS * sum_n Wru[n, kc128, r] --
            # hb-specific selector lhsT x DoubleRow: 8 heads per MM; all 32
            # MMs of one kc accumulate into ONE [128, RP] PSUM bank (rows
            # outside the hb block get +=0), then one scaled ACT copy out
            acc = [sb.tile([128, RPC], BF16, tag=f"acc{k}", name=f"acc{k}")
                   for k in range(4)]
            # v4[:, kc] = row-sum of acc[kc]: with vb these give sum_r S via
            # 5 tiny matmuls, so the tail collective doesn't wait on psS
            v4 = sb.tile([128, 4], BF16, tag="v4")
            v4f = sb.tile([128, 4], F32, tag="v4f")
            hswide = {}

            def headsum_block(kc, hb, gs=range(8)):
                if kc not in hswide:
                    hswide[kc] = pshs.tile([128, RP], F32, tag="hs",
                                           name=f"hs{kc}")
                w = hswide[kc]
                src = wt[(kc, hb)]
                for g in gs:
                    rhs = (src[1][g // 2][:, g % 2] if isinstance(src, tuple)
                           else src[:, g])
                    nc.tensor.matmul(
                        w, sel4[:, hb], rhs,
                        start=(hb == 0 and g == 0), stop=(hb == 3 and g == 7),
                        perf_mode=DR)
                if hb == 3 and gs[-1] == 7:
                    # PSUM -> SBUF with the fp8 dequant scale folded in;
                    # the row-sum reads PSUM directly (parallel, not after)
                    nc.scalar.activation(acc[kc], w[:, :RPC], COPY,
                                         scale=1.0 / QS)
                    nc.vector.tensor_reduce(v4f[:, kc:kc + 1], w[:, :RPC],
                                            axis=mybir.AxisListType.X, op=ADD)
                    nc.vector.tensor_scalar(out=v4[:, kc:kc + 1],
                                            in0=v4f[:, kc:kc + 1],
                                            scalar1=1.0 / QS, scalar2=None,
                                            op0=mybir.AluOpType.mult)

            # kc0-kc2 blocks consume the stream as it lands; fc2 slots in
            # before kc3 so its matmuls hide under the stream tail (the
            # AllReduce completes ~14us before the stream ends)
            for kc in range(3):
                for hb in range(4):
                    headsum_block(kc, hb)

            # ---------- AR readback (per-kc slices) + fc2 (kc-outer) ------
            # relu on the idle DVE (4x mode) instead of ACT; fc2 consumes
            # h1T[kc] as each readback slice lands
            h1T, ps2 = [], []
            MAX = mybir.AluOpType.max
            for kc in range(4):
                hr = sb.tile([128, B], BF16, tag=f"h1r{kc}", name=f"h1r{kc}")
                eng = nc.scalar if kc % 2 == 0 else nc.sync
                eng.dma_start(hr, ar_dout[:, kc, :])
                t = sb.tile([128, B], BF16, tag=f"h1T{kc}", name=f"h1T{kc}")
                nc.vector.tensor_scalar(out=t, in0=hr,
                                        scalar1=b1sb[:, kc:kc + 1],
                                        scalar2=0.0, op0=ADD, op1=MAX)
                h1T.append(t)
            for kc in range(4):
                for ht in range(4):
                    if kc == 0:
                        ps2.append(psfc.tile([128, B], F32, tag="fc",
                                             name=f"ps2_{ht}"))
                    nc.tensor.matmul(
                        ps2[ht],
                        w2sb[:, kc * H + ht * 128:kc * H + (ht + 1) * 128],
                        h1T[kc], start=(kc == 0), stop=(kc == 3))
            hT = []
            for ht in range(4):
                t = sb.tile([128, B], BF16, tag=f"hT{ht}", name=f"hT{ht}")
                nc.scalar.activation(t, ps2[ht], RELU, bias=b2sb[:, ht:ht + 1])
                hT.append(t)

            # the last kc of the stream lands while fc2 runs above
            for hb in range(3):
                headsum_block(3, hb)
            for q in range(4):
                headsum_block(3, 3, gs=range(q * 2, q * 2 + 2))

            ar_in = sb.tile([128, 4], F32, tag="ar_in")
            # S row-sums (cols 2,3) via tiny matmuls on v4/vb and adv_c
            # row-sums (cols 0,1) via vwac/bacsum -- neither waits on the
            # big psS/psC pipelines
            for bt in range(2):
                prs = psfc.tile([128, 1], F32, tag="fc", name=f"prs{bt}")
                nc.tensor.matmul(prs, ones64, vb, start=True, stop=False)
                for kc in range(4):
                    nc.tensor.matmul(prs, hT[kc][:, ts(bt, 128)],
                                     v4[:, kc:kc + 1],
                                     start=False, stop=(kc == 3))
                nc.vector.tensor_copy(ar_in[:, 2 + bt:3 + bt], prs)
            for bt in range(2):
                prs = psfc.tile([128, 1], F32, tag="fc", name=f"prs2_{bt}")
                nc.tensor.matmul(prs, ones1, bacsum, start=True, stop=False)
                for kc in range(4):
                    nc.tensor.matmul(prs, hT[kc][:, ts(bt, 128)],
                                     vwac[:, kc:kc + 1],
                                     start=False, stop=(kc == 3))
                nc.vector.tensor_copy(ar_in[:, bt:bt + 1], prs)

            # single tail AllGather of all four row-sum columns (measured
            # faster than the same-payload AllReduce on this fabric)
            ag_din = dram.tile([128, 4], F32, tag="ag_din")
            ag_dout = dram.tile([NC * 128, 4], F32, tag="ag_dout",
                                addr_space="Shared")
            nc.sync.dma_start(ag_din, ar_in)
            nc.gpsimd.collective_compute(
                "AllGather", mybir.AluOpType.bypass,
                ins=[ag_din.opt()], outs=[ag_dout.opt()],
                replica_groups=[list(range(NC))],
            )

            # ---------- assoc heads + value_c (runs during the AllGather) --
            value_sb = []
            for bt in range(2):
                psA = psw.tile([128, AUG], F32, tag="wide", name=f"psA{bt}")
                nc.tensor.matmul(psA, ones1, augbsb, start=True, stop=False)
                for kc in range(4):
                    nc.tensor.matmul(psA, hT[kc][:, ts(bt, 128)],
                                     augsb[:, kc * AUG:(kc + 1) * AUG],
                                     start=False, stop=(kc == 3))
                psA_sb = sb.tile([128, AUG], F32, tag=f"psAsb{bt}", name=f"psAsb{bt}")
                nc.scalar.copy(psA_sb, psA)
                advs = psA_sb[:, 0:HPC * (M + 1)].rearrange("p (n u) -> p n u", u=M + 1)
                # per-head means on DVE (one 3D reduce) -- much cheaper than
                # HPC activation+read_accumulator pairs on the scalar queue
                r8 = sb.tile([128, HPC], F32, tag=f"r8_{bt}", name=f"r8_{bt}")
                nc.vector.tensor_reduce(
                    r8.rearrange("p (n u) -> p n u", u=1), advs[:, :, 0:M],
                    axis=mybir.AxisListType.X, op=ADD)
                tmp = sb.tile([128, HPC], F32, tag=f"tmpA{bt}", name=f"tmpA{bt}")
                nc.vector.tensor_scalar(out=tmp, in0=r8, scalar1=-1.0 / M,
                                        scalar2=None,
                                        op0=mybir.AluOpType.mult)
                nc.gpsimd.tensor_add(tmp, advs[:, :, M], tmp)
                q = sb.tile([128, HPC * M], F32, tag=f"qA{bt}", name=f"qA{bt}")
                nc.gpsimd.tensor_tensor(
                    out=q.rearrange("p (n m) -> p n m", m=M),
                    in0=advs[:, :, 0:M],
                    in1=tmp.broadcast_to([128, HPC, M]),
                    op=ADD)
                nc.scalar.dma_start(out_assoc[ts(bt, 128), :], q)
                value_sb.append(psA_sb[:, AUG - 1:AUG])

            # ---------- cache head (R-slice) + early row-sums ----------
            adv_c_sb = []
            for bt in range(2):
                psC = pss.tile([128, RPC], F32, tag="s", name=f"psC{bt}")
                nc.tensor.matmul(psC, ones1, bacsb, start=True, stop=False)
                for kc in range(4):
                    nc.tensor.matmul(psC, hT[kc][:, ts(bt, 128)],
                                     wacsb[:, ts(kc, RPC)],
                                     start=False, stop=(kc == 3))
                t = sb.tile([128, RPC], F32, tag=f"advc{bt}", name=f"advc{bt}")
                nc.scalar.activation(t, psC, COPY)
                adv_c_sb.append(t)

            # ---------- S = hT.T @ acc (+ sum_n bru), interleaved by kc ----
            psS = []
            for bt in range(2):
                p = pss.tile([128, RPC], F32, tag="s", name=f"psS{bt}")
                nc.tensor.matmul(p, ones64, brusb, start=True, stop=False)
                psS.append(p)
            for kc in range(4):
                for bt in range(2):
                    nc.tensor.matmul(psS[bt], hT[kc][:, ts(bt, 128)], acc[kc],
                                     start=False, stop=(kc == 3))

            rall = sb.tile([128, NC * 4], F32, tag="rall")
            nc.scalar.dma_start(rall, ag_dout.rearrange("(g p) c -> p g c", p=128))
            rview = bass.AP(rall.tensor, rall.offset,
                            [rall.ap[0], [1, 4], [4, NC]])
            gsum = sb.tile([128, 4], F32, tag="gsum")
            nc.vector.tensor_reduce(gsum, rview, axis=mybir.AxisListType.X, op=ADD)
            negm = sb.tile([128, 4], F32, tag="negm")
            nc.scalar.activation(negm, gsum, COPY, scale=-1.0 / R)
            # qr on DVE straight from PSUM, qc on ACT -- in parallel; each
            # output goes out as ONE DMA (both batch halves)
            # per-half output tiles: half 0's DMA transfer overlaps half 1's
            # compute (qr on DVE, qc on ACT, vm on GpSimd -- all parallel)
            qrh = [sb.tile([128, RPC], BF16, tag=f"qr{bt}", name=f"qr{bt}")
                   for bt in range(2)]
            qch = [sb.tile([128, RPC], BF16, tag=f"qc{bt}", name=f"qc{bt}")
                   for bt in range(2)]
            for bt in range(2):
                nc.vector.tensor_scalar(out=qrh[bt], in0=psS[bt],
                                        scalar1=negm[:, 2 + bt:3 + bt],
                                        scalar2=None, op0=ADD)
                vm = sb.tile([128, 1], F32, tag=f"vm{bt}", name=f"vm{bt}")
                nc.gpsimd.tensor_add(vm, value_sb[bt], negm[:, bt:bt + 1])
                nc.scalar.activation(qch[bt], adv_c_sb[bt], IDENT,
                                     bias=vm, scale=1.0)
                nc.sync.dma_start(out_rec[ts(bt, 128), :], qrh[bt])
                nc.scalar.dma_start(out_cache[ts(bt, 128), :], qch[bt])

    nc.compile()
    return nc


_CACHED = None


def _get_program():
    global _CACHED
    if _CACHED is None:
        _CACHED = build_program()
    return _CACHED


def _fb_quantize_wru(Wru_f):
    """fp8e4 quantization of Wru with error feedback along the head axis.

    The residual of head n's rounding is carried into head n+1, so
    sum_n dequant(q[n]) tracks sum_n Wru[n] to ~1 fp8 ulp per (h, r)
    while each q[n] stays a faithful per-element fp8 image of Wru[n].
    Returns the SCALED fp8 array [64, 512, 4000] (values = Wru*QS).
    """
    q = np.empty((NH, H, R), F8)
    e = np.zeros((H, R), np.float32)
    for n in range(NH):
        t = Wru_f[n] * np.float32(QS) + e
        qn = np.clip(t, -240.0, 240.0).astype(F8)
        e = t - qn.astype(np.float32)
        q[n] = qn
    return q


def make_in_maps(x, W1, b1, W2, b2, Wvc, bvc, Wac, bac, Wvu, bvu, Wau, bau, Wru, bru):
    f = np.float32
    x_bf = np.asarray(x, f).astype(BF)                    # [B, STATE]
    W1_bf = np.asarray(W1, f).astype(BF)                  # [STATE, H]
    W2_bf = np.asarray(W2, f).astype(BF)
    Wac_bf = np.asarray(Wac, f).astype(BF)
    Wru_f = np.asarray(Wru, f)                            # [64, 512, 4000]
    Wau_f = np.asarray(Wau, f)
    Wvu_f = np.asarray(Wvu, f)
    Wvc_f = np.asarray(Wvc, f).reshape(H)
    bau_f = np.asarray(bau, f)
    bvu_f = np.asarray(bvu, f)
    bvc_f = np.asarray(bvc, f).reshape(1)
    bru_bf = np.asarray(bru, f).astype(BF)
    bac_f = np.asarray(bac, f)
    b1_f = np.asarray(b1, f)
    b2_f = np.asarray(b2, f)

    wq = _fb_quantize_wru(Wru_f)                          # [64, 512, 4000] fp8

    # selector: [p, hb*256 + i*128 + m] = (m == hb*32 + p%32)
    selp = np.zeros((128, 4, 2, 128), F8)
    for p in range(128):
        for hb in range(4):
            selp[p, hb, :, hb * 32 + p % 32] = 1.0
    selp = selp.reshape(128, 4 * 256)

    # w2: [h1, h2] -> [p, kc*H + h2]
    w2p = np.ascontiguousarray(
        W2_bf.reshape(4, 128, H).transpose(1, 0, 2)).reshape(128, 4 * H)
    b1p = np.ascontiguousarray(b1_f.reshape(4, 128).T)
    b2p = np.ascontiguousarray(b2_f.reshape(4, 128).T)

    in_maps = []
    for c in range(NC):
        k0 = c * KPC_RAW
        r0 = c * RPC
        h0 = c * HPC
        # xT: [p, kc*B + b] = x[b, k0 + kc*128 + p]
        xs = np.zeros((KPC, B), BF)
        xs[:KPC_RAW] = x_bf[:, k0:k0 + KPC_RAW].T
        xt = np.ascontiguousarray(
            xs.reshape(KCH, 128, B).transpose(1, 0, 2)).reshape(128, KCH * B)
        # w1: [p, kc*H + h] = W1[k0 + kc*128 + p, h]
        w1s = np.zeros((KPC, H), BF)
        w1s[:KPC_RAW] = W1_bf[k0:k0 + KPC_RAW]
        w1p_ = np.ascontiguousarray(
            w1s.reshape(KCH, 128, H).transpose(1, 0, 2)).reshape(128, KCH * H)
        # wac: [p, kc*RPC + r] = Wac[kc*128 + p, r0 + r]
        wacp_ = np.ascontiguousarray(
            Wac_bf[:, r0:r0 + RPC].reshape(4, 128, RPC).transpose(1, 0, 2)
        ).reshape(128, 4 * RPC)
        # wru stream blocks for the DoubleRow selector matmuls:
        # [kc, hb, n4*32+h32, ((g*2+i)*RP + r)] = wq[8g+4i+n4, kc*128+hb*32+h32, r0+r]
        ws = wq[:, :, r0:r0 + RPC]                        # [64, 512, 500]
        a = ws.reshape(8, 2, 4, 4, 4, 32, RPC)            # [g, i, n4, kc, hb, h32, r]
        a = a.transpose(3, 4, 2, 5, 0, 1, 6)              # [kc, hb, n4, h32, g, i, r]
        wru_p = np.zeros((4, 4, 4, 32, 8, 2, RP), F8)
        wru_p[..., :RPC] = a
        wru_p = wru_p.reshape(4, 4, 128, 16 * RP)
        # aug weights: [k, n*(M+1)+m | n*(M+1)+M | 88]
        aug_full = np.empty((H, AUG), f)
        aug_full[:, 0:HPC * (M + 1)] = np.concatenate(
            [Wau_f[h0:h0 + HPC].transpose(1, 0, 2),            # [H, 8, 10]
             Wvu_f[h0:h0 + HPC].T[:, :, None]], axis=2         # [H, 8, 1]
        ).reshape(H, HPC * (M + 1))
        aug_full[:, AUG - 1] = Wvc_f
        augp_ = np.ascontiguousarray(
            aug_full.astype(BF).reshape(4, 128, AUG).transpose(1, 0, 2)
        ).reshape(128, 4 * AUG)
        augb_ = np.empty((1, AUG), f)
        augb_[0, 0:HPC * (M + 1)] = np.concatenate(
            [bau_f[h0:h0 + HPC], bvu_f[h0:h0 + HPC, None]], axis=1
        ).reshape(HPC * (M + 1))
        augb_[0, AUG - 1] = bvc_f[0]
        m = {
            "xt": xt,
            "w1": w1p_,
            "b1p": b1p,
            "w2p": w2p,
            "b2p": b2p,
            "wacp": wacp_,
            "bacp": np.ascontiguousarray(bac_f[None, r0:r0 + RPC]).astype(BF),
            "wrup": wru_p,
            "selp": selp,
            "brup": np.ascontiguousarray(bru_bf[:, r0:r0 + RPC]),
            "augp": augp_,
            "augb": augb_.astype(BF),
        }
        in_maps.append(m)
    return in_maps


def assemble(results):
    q = np.empty((B, 2 * R + NH * M), np.float32)
    for c in range(NC):
        r0 = c * RPC
        a0 = c * HPC * M
        q[:, r0:r0 + RPC] = results[c]["out_cache"]
        q[:, R + r0:R + r0 + RPC] = results[c]["out_rec"]
        q[:, 2 * R + a0:2 * R + a0 + HPC * M] = results[c]["out_assoc"]
    return q


def run(in_maps, **kw):
    nc = _get_program()
    return bass_utils.run_bass_kernel_spmd(nc, in_maps, core_ids=list(range(NC)), **kw)


def kernel(**inputs):
    in_maps = make_in_maps(**{k: np.asarray(v) for k, v in inputs.items()})
    res = run(in_maps)
    return assemble(res.results)
